# revision 19
# baseline (speedup 1.0000x reference)
"""NT-Xent loss kernel for 8 TRN2 NeuronCores (Bass/Tile).

Computes: reps = l2norm(concat(z_i, z_j)); sim = reps @ reps.T / T;
e = exp(sim); lse_i = logsumexp over off-diagonal e-row; pos_i = e[i, i+-B];
loss = mean(lse - pos).

Key numerical identity exploited here: the "logits" handed to the CE are
e = exp(sim/T), which span [e^-14, e^14].  logsumexp over such doubly-
exponential values collapses (in fp32, exactly as the reference computes
it) to the max term: lse_i = max_j e_ij + log(1 + eps) where the eps terms
vanish below fp32 precision unless a row has two sims within ~0.0004 of
its max.  Verified against the fp64 reference on these inputs: rel err
2e-5 (tolerance 2e-2).  So lse_i = exp(max_j sim_ij / T) and the kernel
only needs a row-max of sim — no full-matrix exp passes at all.

Strategy (data-parallel rows, fully fused on-chip; primitives verified on
this HW — fused DVE reduce ops crash the exec unit, TensorReduce has no
fp16 fast mode, TensorTensor fp16 runs at 2x, GpSimd cannot touch PSUM):
  - Host: l2-normalize, transpose to [D=128, 2B=16384], cast fp16.
  - Each core c gets a column-ROTATED copy (roll by -c*2048) so its own
    2048 row-vectors sit in rotated chunk 0: diagonal/positive columns are
    compile-time constants -> one SPMD program.
  - Per 128-row block: 10 pieces of 1536 cols + 1 piece of 1024 (PSUM =
    [128,1536]x2 + [128,1024]x1 = exactly 8 banks, 3 pieces in flight).
    Row-max per piece via two balanced paths:
      direct (DVE): reduce_max straight off PSUM fp32 (2x1536 + the 1024)
      staged (ACT+DVE): ACT copies pairs of 1536-pieces into halves of a
        [128,3072] fp16 tile (~1.59us/piece, wide stages amortize the
        ~310ns/instr ACT overhead); an in-place fp16 tensor_tensor max
        accumulator chain folds pairs at 2x DVE speed; final half-fold +
        [128,1536] reduce per block.
    2+1 direct + 8 staged per block balances DVE (~202us) vs ACT (~204us).
  - Diagonal killed ON THE PE: the self [128,128] window gets an extra
    accumulated matmul of I.T @ (-30000 stripe) — zero consumer cost.
  - Positives pos_i = r_i . r_{i+B}: one fp16 elementwise multiply of
    rotated chunks 0 and 4, partition-summed by a ones-column matmul,
    exp'd by [1,1024] ACT ops.
  - End: block maxes -> ACT exp(max/T) with accum_out row-sums.
    Host: loss = (sum(lse) - sum(pos)) / 16384.
"""

import os
import numpy as np

TEMP = 0.07
B = 8192
D = 128
N = 2 * B            # 16384 rows/cols of sim
NCORES = 8
ROWS_PER_CORE = N // NCORES   # 2048
BLKS = ROWS_PER_CORE // 128   # 16 row-blocks per core
OUT_LEN = 512                 # [128, 4] f32: lse sums, pos partials x2, pad

# pieces per block row: (start_col, width); widths tile 16384 = 10*1536+1024
PIECES = [(i * 1536, 1536) for i in range(10)] + [(15360, 1024)]
DIRECT_PS = (0, 5, 10)        # pieces reduced straight off PSUM
STAGED_PS = [i for i in range(11) if i not in DIRECT_PS]  # 8, pairs up

_cache = {}


def build_nc():
    """Build the SPMD Bass program (identical for all cores)."""
    import concourse.bacc as bacc
    import concourse.bass as bass
    import concourse.mybir as mybir
    import concourse.tile as tile

    f32 = mybir.dt.float32
    f16 = mybir.dt.float16
    AF = mybir.ActivationFunctionType
    ALU = mybir.AluOpType

    nc = bacc.Bacc(
        "TRN2",
        target_bir_lowering=False,
        debug=False,
        num_devices=NCORES,
    )

    zt_d = nc.dram_tensor("zt", [D, N], f16, kind="ExternalInput").ap()
    eyew_d = nc.dram_tensor("eyew", [128, 128], f16, kind="ExternalInput").ap()
    m4_d = nc.dram_tensor("m4", [128, 2048], f16, kind="ExternalInput").ap()
    wcol_d = nc.dram_tensor("wcol", [128, 128], f16, kind="ExternalInput").ap()
    out_d = nc.dram_tensor("out", [OUT_LEN], f32, kind="ExternalOutput").ap()

    SLOTS = 4  # mstage slots per block: 3 direct + 1 staged-chain

    with tile.TileContext(nc) as tc:
        with (
            tc.tile_pool(name="rpool", bufs=8) as rpool,
            tc.tile_pool(name="cpool", bufs=1) as cpool,
            tc.tile_pool(name="apool", bufs=2) as apool,
            tc.tile_pool(name="stpool", bufs=2) as stpool,
            tc.tile_pool(name="psA", bufs=2, space=bass.MemorySpace.PSUM) as psA,
            tc.tile_pool(name="psB", bufs=1, space=bass.MemorySpace.PSUM) as psB,
        ):
            # ---- consts first (tiny), then R chunks sequentially so chunk q
            # lands at ~1.4*(q+1) us and stage-1 matmuls can start early ----
            eyew = cpool.tile([128, 128], f16, tag="eyew")
            nc.sync.dma_start(eyew[:], eyew_d[:])
            m4 = cpool.tile([128, 2048], f16, tag="m4")
            nc.sync.dma_start(m4[:], m4_d[:])
            wcol = cpool.tile([128, 128], f16, tag="wcol")
            nc.sync.dma_start(wcol[:], wcol_d[:])
            R = []
            for q in range(8):
                rq = rpool.tile([D, 2048], f16, tag="rchunk")
                nc.sync.dma_start(rq[:], zt_d[:, q * 2048:(q + 1) * 2048])
                R.append(rq)

            mstage = cpool.tile([128, BLKS * SLOTS], f32, tag="mstage")
            nc.vector.memset(mstage[:], -1e30)
            posP = cpool.tile([128, 2048], f16, tag="posP")
            outstage = cpool.tile([128, 4], f32, tag="outstage")
            nc.vector.memset(outstage[:], 0.0)

            # emission order: stage 1 interleaves blocks 0-1 piece-major to
            # hide the R load; the rest is block-major (weight reuse).
            order = []
            for p in range(len(PIECES)):
                for b in (0, 1):
                    order.append((b, p))
            for b in range(2, BLKS):
                for p in range(len(PIECES)):
                    order.append((b, p))

            accs = {}       # b -> [128,3072] fp16 accumulator tile
            halves = {}     # b -> pending [128,3072] tile with one half staged
            slot = [0] * BLKS

            def rhs_seg(gcol):
                q, off = gcol // 2048, gcol % 2048
                return R[q][:, off:off + 512]

            for (b, p) in order:
                start, width = PIECES[p]
                pool = psB if width == 1024 else psA
                ps = pool.tile([128, width], f32, tag="ps")
                lhsT = R[0][:, b * 128:(b + 1) * 128]
                mask_gseg = (b * 128) // 512  # global 512-seg of diag window
                for t in range(width // 512):
                    gcol = start + t * 512
                    is_mask_seg = gcol // 512 == mask_gseg
                    nc.tensor.matmul(
                        ps[:, t * 512:(t + 1) * 512],
                        lhsT,
                        rhs_seg(gcol),
                        start=True,
                        stop=not is_mask_seg,
                    )
                    if is_mask_seg:
                        # accumulate -30000 onto the self-diagonal window:
                        # I.T @ stripe lands exactly on sim[p, b*128+p]
                        k = ((b * 128) % 512) // 128
                        nc.tensor.matmul(
                            ps[:, t * 512:(t + 1) * 512],
                            eyew[:],
                            m4[:, k * 512:(k + 1) * 512],
                            start=False,
                            stop=True,
                        )
                if p in DIRECT_PS:
                    nc.vector.reduce_max(
                        mstage[:, b * SLOTS + slot[b]:b * SLOTS + slot[b] + 1],
                        ps[:],
                        axis=mybir.AxisListType.X,
                    )
                    slot[b] += 1
                else:
                    if b in halves:
                        pair = halves.pop(b)
                        nc.scalar.activation(
                            pair[:, 1536:3072], ps[:], AF.Copy, scale=1.0
                        )
                        if b not in accs:
                            accs[b] = pair
                        else:
                            nc.vector.tensor_tensor(
                                accs[b][:], accs[b][:], pair[:], op=ALU.max
                            )
                    else:
                        tile_src = apool if b not in accs else stpool
                        pair = tile_src.tile([128, 3072], f16, tag="pair",
                                             name=f"pair{b}_{p}")
                        nc.scalar.activation(
                            pair[:, 0:1536], ps[:], AF.Copy, scale=1.0
                        )
                        halves[b] = pair
                if p == len(PIECES) - 1:
                    acc = accs.pop(b)
                    nc.vector.tensor_tensor(
                        acc[:, 0:1536], acc[:, 0:1536], acc[:, 1536:3072],
                        op=ALU.max,
                    )
                    nc.vector.reduce_max(
                        mstage[:, b * SLOTS + slot[b]:b * SLOTS + slot[b] + 1],
                        acc[:, 0:1536],
                        axis=mybir.AxisListType.X,
                    )
                    slot[b] += 1
                if (b, p) == (0, 6):
                    # positives: elementwise r_i * r_{i+B} (cols 0..2047 vs
                    # 8192..10239 of the rotated layout)
                    nc.vector.tensor_tensor(posP[:], R[0][:], R[4][:], op=ALU.mult)

            # ---- positives: partition-sum via ones-column matmul, then exp
            for half in range(2):
                ps_pos = psB.tile([128, 1024], f32, tag="ps")
                for t in range(2):
                    off = half * 1024 + t * 512
                    nc.tensor.matmul(
                        ps_pos[:, t * 512:(t + 1) * 512],
                        wcol[:],
                        posP[:, off:off + 512],
                        start=True,
                        stop=True,
                    )
                pxp = cpool.tile([1, 1024], f32, tag=f"posexp{half}",
                                 name=f"posexp{half}")
                nc.scalar.activation(
                    pxp[:], ps_pos[0:1, :], AF.Exp, scale=1.0 / TEMP,
                    accum_out=outstage[0:1, 1 + half:2 + half],
                )

            # ---- finalize: block maxes -> lse = exp(max/T) ----
            bmax = cpool.tile([128, BLKS], f32, tag="bmax")
            nc.vector.reduce_max(
                bmax[:],
                mstage[:].rearrange("p (b s) -> p b s", s=SLOTS),
                axis=mybir.AxisListType.X,
            )
            lscr = cpool.tile([128, BLKS], f32, tag="lscr")
            nc.scalar.activation(
                lscr[:], bmax[:], AF.Exp, scale=1.0 / TEMP,
                accum_out=outstage[:, 0:1],
            )
            nc.sync.dma_start(
                out_d.rearrange("(p o) -> p o", o=4),
                outstage[:],
            )

    nc.compile()
    return nc


def make_in_maps(z_i: np.ndarray, z_j: np.ndarray):
    Z = np.concatenate([np.asarray(z_i), np.asarray(z_j)], axis=0).astype(np.float32)
    nrm = np.linalg.norm(Z, axis=1, keepdims=True)
    R = (Z / np.maximum(nrm, 1e-12)).astype(np.float32)
    RT = np.ascontiguousarray(R.T).astype(np.float16)  # [128, 16384]
    eyew = np.eye(128, dtype=np.float16)
    m4 = np.zeros((128, 2048), dtype=np.float16)
    for k in range(4):
        for p in range(128):
            m4[p, 512 * k + 128 * k + p] = -30000.0
    wcol = np.zeros((128, 128), dtype=np.float16)
    wcol[:, 0] = 1.0
    in_maps = []
    for c in range(NCORES):
        zt = np.ascontiguousarray(np.roll(RT, -c * ROWS_PER_CORE, axis=1))
        in_maps.append({"zt": zt, "eyew": eyew, "m4": m4, "wcol": wcol})
    return in_maps


def kernel(z_i: np.ndarray, z_j: np.ndarray) -> np.ndarray:
    from concourse.bass_utils import run_bass_kernel_spmd

    if "nc" not in _cache:
        _cache["nc"] = build_nc()
    nc = _cache["nc"]

    in_maps = make_in_maps(z_i, z_j)
    res = run_bass_kernel_spmd(
        nc,
        in_maps,
        core_ids=list(range(NCORES)),
        trace=bool(int(os.environ.get("NTX_TRACE", "0"))),
    )
    _cache["last_result"] = res

    lse_sum = 0.0
    pos_sum = 0.0
    for c in range(NCORES):
        out = res.results[c]["out"].astype(np.float64).reshape(128, 4)
        lse_sum += out[:, 0].sum()
        pos_sum += out[:, 1].sum() + out[:, 2].sum()
    loss = (lse_sum - pos_sum) / float(N)
    return np.float32(loss)


# revision 20
# speedup vs baseline: 1.0056x; 1.0056x over previous
"""NT-Xent loss kernel for 8 TRN2 NeuronCores (Bass/Tile).

Computes: reps = l2norm(concat(z_i, z_j)); sim = reps @ reps.T / T;
e = exp(sim); lse_i = logsumexp over off-diagonal e-row; pos_i = e[i, i+-B];
loss = mean(lse - pos).

Key numerical identity exploited here: the "logits" handed to the CE are
e = exp(sim/T), which span [e^-14, e^14].  logsumexp over such doubly-
exponential values collapses (in fp32, exactly as the reference computes
it) to the max term: lse_i = max_j e_ij + log(1 + eps) where the eps terms
vanish below fp32 precision unless a row has two sims within ~0.0004 of
its max.  Verified against the fp64 reference on these inputs: rel err
2e-5 (tolerance 2e-2).  So lse_i = exp(max_j sim_ij / T) and the kernel
only needs a row-max of sim — no full-matrix exp passes at all.

Strategy (data-parallel rows, fully fused on-chip; primitives verified on
this HW — fused DVE reduce ops crash the exec unit, TensorReduce has no
fp16 fast mode, TensorTensor fp16 runs at 2x, GpSimd cannot touch PSUM):
  - Host: l2-normalize, transpose to [D=128, 2B=16384], cast fp16.
  - Each core c gets a column-ROTATED copy (roll by -c*2048) so its own
    2048 row-vectors sit in rotated chunk 0: diagonal/positive columns are
    compile-time constants -> one SPMD program.
  - Per 128-row block: 10 pieces of 1536 cols + 1 piece of 1024 (PSUM =
    [128,1536]x2 + [128,1024]x1 = exactly 8 banks, 3 pieces in flight).
    Row-max per piece via two balanced paths:
      direct (DVE): reduce_max straight off PSUM fp32 (2x1536 + the 1024)
      staged (ACT+DVE): ACT copies pairs of 1536-pieces into halves of a
        [128,3072] fp16 tile (~1.59us/piece, wide stages amortize the
        ~310ns/instr ACT overhead); an in-place fp16 tensor_tensor max
        accumulator chain folds pairs at 2x DVE speed; final half-fold +
        [128,1536] reduce per block.
    2+1 direct + 8 staged per block balances DVE (~202us) vs ACT (~204us).
  - Diagonal killed ON THE PE: the self [128,128] window gets an extra
    accumulated matmul of I.T @ (-30000 stripe) — zero consumer cost.
  - Positives pos_i = r_i . r_{i+B}: one fp16 elementwise multiply of
    rotated chunks 0 and 4, partition-summed by a ones-column matmul,
    exp'd by [1,1024] ACT ops.
  - End: block maxes -> ACT exp(max/T) with accum_out row-sums.
    Host: loss = (sum(lse) - sum(pos)) / 16384.
"""

import os
import numpy as np

TEMP = 0.07
B = 8192
D = 128
N = 2 * B            # 16384 rows/cols of sim
NCORES = 8
ROWS_PER_CORE = N // NCORES   # 2048
BLKS = ROWS_PER_CORE // 128   # 16 row-blocks per core
OUT_LEN = 512                 # [128, 4] f32: lse sums, pos partials x2, pad

# pieces per block row: (start_col, width); widths tile 16384 = 10*1536+1024
PIECES = [(i * 1536, 1536) for i in range(10)] + [(15360, 1024)]
DIRECT_PS = (2, 7, 10)        # pieces reduced straight off PSUM
STAGED_PS = [i for i in range(11) if i not in DIRECT_PS]  # 8, pairs up

_cache = {}


def build_nc():
    """Build the SPMD Bass program (identical for all cores)."""
    import concourse.bacc as bacc
    import concourse.bass as bass
    import concourse.mybir as mybir
    import concourse.tile as tile

    f32 = mybir.dt.float32
    f16 = mybir.dt.float16
    AF = mybir.ActivationFunctionType
    ALU = mybir.AluOpType

    nc = bacc.Bacc(
        "TRN2",
        target_bir_lowering=False,
        debug=False,
        num_devices=NCORES,
    )

    zt_d = nc.dram_tensor("zt", [D, N], f16, kind="ExternalInput").ap()
    eyew_d = nc.dram_tensor("eyew", [128, 128], f16, kind="ExternalInput").ap()
    m4_d = nc.dram_tensor("m4", [128, 2048], f16, kind="ExternalInput").ap()
    wcol_d = nc.dram_tensor("wcol", [128, 128], f16, kind="ExternalInput").ap()
    out_d = nc.dram_tensor("out", [OUT_LEN], f32, kind="ExternalOutput").ap()

    SLOTS = 4  # mstage slots per block: 3 direct + 1 staged-chain

    with tile.TileContext(nc) as tc:
        with (
            tc.tile_pool(name="rpool", bufs=8) as rpool,
            tc.tile_pool(name="cpool", bufs=1) as cpool,
            tc.tile_pool(name="apool", bufs=2) as apool,
            tc.tile_pool(name="stpool", bufs=3) as stpool,
            tc.tile_pool(name="psA", bufs=2, space=bass.MemorySpace.PSUM) as psA,
            tc.tile_pool(name="psB", bufs=1, space=bass.MemorySpace.PSUM) as psB,
        ):
            # ---- consts first (tiny), then R chunks sequentially so chunk q
            # lands at ~1.4*(q+1) us and stage-1 matmuls can start early ----
            eyew = cpool.tile([128, 128], f16, tag="eyew")
            nc.sync.dma_start(eyew[:], eyew_d[:])
            m4 = cpool.tile([128, 2048], f16, tag="m4")
            nc.sync.dma_start(m4[:], m4_d[:])
            wcol = cpool.tile([128, 128], f16, tag="wcol")
            nc.sync.dma_start(wcol[:], wcol_d[:])
            R = []
            for q in range(8):
                rq = rpool.tile([D, 2048], f16, tag="rchunk")
                nc.sync.dma_start(rq[:], zt_d[:, q * 2048:(q + 1) * 2048])
                R.append(rq)

            mstage = cpool.tile([128, BLKS * SLOTS], f32, tag="mstage")
            nc.vector.memset(mstage[:], -1e30)
            posP = cpool.tile([128, 2048], f16, tag="posP")
            outstage = cpool.tile([128, 4], f32, tag="outstage")
            nc.vector.memset(outstage[:], 0.0)

            # emission order: stage 1 interleaves blocks 0-1 piece-major to
            # hide the R load; the rest is block-major (weight reuse).
            order = []
            for p in range(len(PIECES)):
                for b in (0, 1):
                    order.append((b, p))
            for b in range(2, BLKS):
                for p in range(len(PIECES)):
                    order.append((b, p))

            accs = {}       # b -> [128,3072] fp16 accumulator tile
            halves = {}     # b -> pending [128,3072] tile with one half staged
            slot = [0] * BLKS

            def rhs_seg(gcol):
                q, off = gcol // 2048, gcol % 2048
                return R[q][:, off:off + 512]

            for (b, p) in order:
                start, width = PIECES[p]
                pool = psB if width == 1024 else psA
                ps = pool.tile([128, width], f32, tag="ps")
                lhsT = R[0][:, b * 128:(b + 1) * 128]
                mask_gseg = (b * 128) // 512  # global 512-seg of diag window
                for t in range(width // 512):
                    gcol = start + t * 512
                    is_mask_seg = gcol // 512 == mask_gseg
                    nc.tensor.matmul(
                        ps[:, t * 512:(t + 1) * 512],
                        lhsT,
                        rhs_seg(gcol),
                        start=True,
                        stop=not is_mask_seg,
                    )
                    if is_mask_seg:
                        # accumulate -30000 onto the self-diagonal window:
                        # I.T @ stripe lands exactly on sim[p, b*128+p]
                        k = ((b * 128) % 512) // 128
                        nc.tensor.matmul(
                            ps[:, t * 512:(t + 1) * 512],
                            eyew[:],
                            m4[:, k * 512:(k + 1) * 512],
                            start=False,
                            stop=True,
                        )
                if p in DIRECT_PS:
                    nc.vector.reduce_max(
                        mstage[:, b * SLOTS + slot[b]:b * SLOTS + slot[b] + 1],
                        ps[:],
                        axis=mybir.AxisListType.X,
                    )
                    slot[b] += 1
                else:
                    if b in halves:
                        pair = halves.pop(b)
                        nc.scalar.activation(
                            pair[:, 1536:3072], ps[:], AF.Copy, scale=1.0
                        )
                        if b not in accs:
                            accs[b] = pair
                        else:
                            nc.vector.tensor_tensor(
                                accs[b][:], accs[b][:], pair[:], op=ALU.max
                            )
                    else:
                        tile_src = apool if b not in accs else stpool
                        pair = tile_src.tile([128, 3072], f16, tag="pair",
                                             name=f"pair{b}_{p}")
                        nc.scalar.activation(
                            pair[:, 0:1536], ps[:], AF.Copy, scale=1.0
                        )
                        halves[b] = pair
                if p == len(PIECES) - 1:
                    acc = accs.pop(b)
                    nc.vector.tensor_tensor(
                        acc[:, 0:1536], acc[:, 0:1536], acc[:, 1536:3072],
                        op=ALU.max,
                    )
                    nc.vector.reduce_max(
                        mstage[:, b * SLOTS + slot[b]:b * SLOTS + slot[b] + 1],
                        acc[:, 0:1536],
                        axis=mybir.AxisListType.X,
                    )
                    slot[b] += 1
                if (b, p) == (0, 6):
                    # positives: elementwise r_i * r_{i+B} (cols 0..2047 vs
                    # 8192..10239 of the rotated layout)
                    nc.vector.tensor_tensor(posP[:], R[0][:], R[4][:], op=ALU.mult)

            # ---- positives: partition-sum via ones-column matmul, then exp
            for half in range(2):
                ps_pos = psB.tile([128, 1024], f32, tag="ps")
                for t in range(2):
                    off = half * 1024 + t * 512
                    nc.tensor.matmul(
                        ps_pos[:, t * 512:(t + 1) * 512],
                        wcol[:],
                        posP[:, off:off + 512],
                        start=True,
                        stop=True,
                    )
                pxp = cpool.tile([1, 1024], f32, tag=f"posexp{half}",
                                 name=f"posexp{half}")
                nc.scalar.activation(
                    pxp[:], ps_pos[0:1, :], AF.Exp, scale=1.0 / TEMP,
                    accum_out=outstage[0:1, 1 + half:2 + half],
                )

            # ---- finalize: block maxes -> lse = exp(max/T) ----
            bmax = cpool.tile([128, BLKS], f32, tag="bmax")
            nc.vector.reduce_max(
                bmax[:],
                mstage[:].rearrange("p (b s) -> p b s", s=SLOTS),
                axis=mybir.AxisListType.X,
            )
            lscr = cpool.tile([128, BLKS], f32, tag="lscr")
            nc.scalar.activation(
                lscr[:], bmax[:], AF.Exp, scale=1.0 / TEMP,
                accum_out=outstage[:, 0:1],
            )
            nc.sync.dma_start(
                out_d.rearrange("(p o) -> p o", o=4),
                outstage[:],
            )

    nc.compile()
    return nc


def make_in_maps(z_i: np.ndarray, z_j: np.ndarray):
    Z = np.concatenate([np.asarray(z_i), np.asarray(z_j)], axis=0).astype(np.float32)
    nrm = np.linalg.norm(Z, axis=1, keepdims=True)
    R = (Z / np.maximum(nrm, 1e-12)).astype(np.float32)
    RT = np.ascontiguousarray(R.T).astype(np.float16)  # [128, 16384]
    eyew = np.eye(128, dtype=np.float16)
    m4 = np.zeros((128, 2048), dtype=np.float16)
    for k in range(4):
        for p in range(128):
            m4[p, 512 * k + 128 * k + p] = -30000.0
    wcol = np.zeros((128, 128), dtype=np.float16)
    wcol[:, 0] = 1.0
    in_maps = []
    for c in range(NCORES):
        zt = np.ascontiguousarray(np.roll(RT, -c * ROWS_PER_CORE, axis=1))
        in_maps.append({"zt": zt, "eyew": eyew, "m4": m4, "wcol": wcol})
    return in_maps


def kernel(z_i: np.ndarray, z_j: np.ndarray) -> np.ndarray:
    from concourse.bass_utils import run_bass_kernel_spmd

    if "nc" not in _cache:
        _cache["nc"] = build_nc()
    nc = _cache["nc"]

    in_maps = make_in_maps(z_i, z_j)
    res = run_bass_kernel_spmd(
        nc,
        in_maps,
        core_ids=list(range(NCORES)),
        trace=bool(int(os.environ.get("NTX_TRACE", "0"))),
    )
    _cache["last_result"] = res

    lse_sum = 0.0
    pos_sum = 0.0
    for c in range(NCORES):
        out = res.results[c]["out"].astype(np.float64).reshape(128, 4)
        lse_sum += out[:, 0].sum()
        pos_sum += out[:, 1].sum() + out[:, 2].sum()
    loss = (lse_sum - pos_sum) / float(N)
    return np.float32(loss)


# revision 21
# speedup vs baseline: 1.1717x; 1.1652x over previous
"""NT-Xent loss kernel for 8 TRN2 NeuronCores (Bass/Tile).

Computes: reps = l2norm(concat(z_i, z_j)); sim = reps @ reps.T / T;
e = exp(sim); lse_i = logsumexp over off-diagonal e-row; pos_i = e[i, i+-B];
loss = mean(lse - pos).

Key numerical identity exploited here: the "logits" handed to the CE are
e = exp(sim/T), which span [e^-14, e^14].  logsumexp over such doubly-
exponential values collapses (in fp32, exactly as the reference computes
it) to the max term: lse_i = max_j e_ij + log(1 + eps) where the eps terms
vanish below fp32 precision unless a row has two sims within ~0.0004 of
its max.  Verified against the fp64 reference on these inputs: rel err
2e-5 (tolerance 2e-2).  So lse_i = exp(max_j sim_ij / T) and the kernel
only needs a row-max of sim — no full-matrix exp passes at all.

Strategy (data-parallel rows, fully fused on-chip; primitives verified on
this HW — fused DVE reduce ops crash the exec unit, TensorReduce has no
fp16 fast mode, TensorTensor fp16 runs at 2x, GpSimd cannot touch PSUM):
  - Host: l2-normalize, transpose to [D=128, 2B=16384], cast fp16.
  - Each core c gets a column-ROTATED copy (roll by -c*2048) so its own
    2048 row-vectors sit in rotated chunk 0: diagonal/positive columns are
    compile-time constants -> one SPMD program.
  - Per 128-row block: 16 pieces of 1024 cols; 2 fp16 matmuls [128,512]
    -> PSUM ([128,1024] tiles x4 so four pieces are in flight and the
    DVE/ACT consumers overlap).  Row-max per piece via two balanced paths:
      direct (DVE): reduce_max straight off PSUM fp32     (~1.20us/piece)
      staged (ACT+DVE): ACT copies pairs of pieces into halves of a
        [128,2048] fp16 tile (~1.11us/piece); an in-place fp16
        tensor_tensor max accumulator chain folds staged pairs at 2x DVE
        speed (~1.21us/fold of 2 pieces); final half-fold + [128,1024]
        reduce per block.
    ~4.4 direct + ~11.6 staged per block balances DVE against ACT.
  - Diagonal killed ON THE PE: the self [128,128] window gets an extra
    accumulated matmul of I.T @ (-30000 stripe) — zero consumer cost.
  - Positives pos_i = r_i . r_{i+B}: one fp16 elementwise multiply of
    rotated chunks 0 and 4, partition-summed by a ones-column matmul,
    exp'd by [1,1024] ACT ops.
  - End: block maxes -> ACT exp(max/T) with accum_out row-sums.
    Host: loss = (sum(lse) - sum(pos)) / 16384.
"""

import os
import numpy as np

TEMP = 0.07
B = 8192
D = 128
N = 2 * B            # 16384 rows/cols of sim
NCORES = 8
ROWS_PER_CORE = N // NCORES   # 2048
BLKS = ROWS_PER_CORE // 128   # 16 row-blocks per core
PIECE = 1024                  # PSUM piece width
NPIECE = N // PIECE           # 16 pieces per block row
OUT_LEN = 512                 # [128, 4] f32: lse sums, pos partials x2, pad

_cache = {}

# measured: all-4-direct -> ACT 219.7/DVE 210.9; all-5-direct -> ACT
# 201.2/DVE 219.5; balance lands at ~70-73 direct pieces per core
D5_BLOCKS = {2, 4, 7, 10, 12, 15}


def _direct_ps(b):
    return (0, 1, 5, 9, 13) if b in D5_BLOCKS else (0, 1, 6, 11)


def build_nc():
    """Build the SPMD Bass program (identical for all cores)."""
    import concourse.bacc as bacc
    import concourse.bass as bass
    import concourse.mybir as mybir
    import concourse.tile as tile

    f32 = mybir.dt.float32
    f16 = mybir.dt.float16
    AF = mybir.ActivationFunctionType
    ALU = mybir.AluOpType

    nc = bacc.Bacc(
        "TRN2",
        target_bir_lowering=False,
        debug=False,
        num_devices=NCORES,
    )

    zt_d = nc.dram_tensor("zt", [D, N], f16, kind="ExternalInput").ap()
    eyew_d = nc.dram_tensor("eyew", [128, 128], f16, kind="ExternalInput").ap()
    m4_d = nc.dram_tensor("m4", [128, 2048], f16, kind="ExternalInput").ap()
    wcol_d = nc.dram_tensor("wcol", [128, 128], f16, kind="ExternalInput").ap()
    out_d = nc.dram_tensor("out", [OUT_LEN], f32, kind="ExternalOutput").ap()

    SLOTS = 6  # mstage slots per block: <=5 direct + 1 staged-chain

    with tile.TileContext(nc) as tc:
        with (
            tc.tile_pool(name="rpool", bufs=8) as rpool,
            tc.tile_pool(name="cpool", bufs=1) as cpool,
            tc.tile_pool(name="apool", bufs=2) as apool,
            tc.tile_pool(name="stpool", bufs=3) as stpool,
            tc.tile_pool(name="s1pool", bufs=2) as s1pool,
            tc.tile_pool(name="psum", bufs=4, space=bass.MemorySpace.PSUM) as psumpool,
        ):
            # ---- consts first (tiny), then R chunks sequentially so chunk q
            # lands at ~1.4*(q+1) us and stage-1 matmuls can start early ----
            eyew = cpool.tile([128, 128], f16, tag="eyew")
            nc.sync.dma_start(eyew[:], eyew_d[:])
            m4 = cpool.tile([128, 2048], f16, tag="m4")
            nc.sync.dma_start(m4[:], m4_d[:])
            wcol = cpool.tile([128, 128], f16, tag="wcol")
            nc.sync.dma_start(wcol[:], wcol_d[:])
            R = []
            for q in range(8):
                rq = rpool.tile([D, 2048], f16, tag="rchunk")
                nc.sync.dma_start(rq[:], zt_d[:, q * 2048:(q + 1) * 2048])
                R.append(rq)

            mstage = cpool.tile([128, BLKS * SLOTS], f32, tag="mstage")
            nc.vector.memset(mstage[:], -1e30)
            posP = cpool.tile([128, 2048], f16, tag="posP")
            outstage = cpool.tile([128, 4], f32, tag="outstage")
            nc.vector.memset(outstage[:], 0.0)

            # emission order: stage 1 interleaves blocks 0-1 piece-major to
            # hide the R load; the rest is block-major (weight reuse).
            order = []
            for p in range(NPIECE):
                for b in (0, 1):
                    order.append((b, p))
            for b in range(2, BLKS):
                for p in range(NPIECE):
                    order.append((b, p))

            accs = {}       # b -> [128,2048] fp16 accumulator tile
            halves = {}     # b -> pending [128,2048] tile with one half staged
            odd1 = {}       # b -> leftover [128,1024] staged tile
            slot = [0] * BLKS

            def rhs_slice(p, t):
                q, half = p // 2, p % 2
                off = half * 1024 + t * 512
                return R[q][:, off:off + 512]

            for (b, p) in order:
                direct_ps = _direct_ps(b)
                n_staged = NPIECE - len(direct_ps)
                ps = psumpool.tile([128, PIECE], f32, tag="ps")
                lhsT = R[0][:, b * 128:(b + 1) * 128]
                mask_p = (b * 128) // PIECE
                mask_off = (b * 128) % PIECE
                for t in range(2):
                    is_mask_seg = p == mask_p and t == mask_off // 512
                    nc.tensor.matmul(
                        ps[:, t * 512:(t + 1) * 512],
                        lhsT,
                        rhs_slice(p, t),
                        start=True,
                        stop=not is_mask_seg,
                    )
                    if is_mask_seg:
                        # accumulate -30000 onto the self-diagonal window:
                        # I.T @ stripe lands exactly on sim[p, b*128+p]
                        v = (b * 128) % 512
                        k = v // 128
                        nc.tensor.matmul(
                            ps[:, t * 512:(t + 1) * 512],
                            eyew[:],
                            m4[:, k * 512:(k + 1) * 512],
                            start=False,
                            stop=True,
                        )
                if p in direct_ps:
                    nc.vector.reduce_max(
                        mstage[:, b * SLOTS + slot[b]:b * SLOTS + slot[b] + 1],
                        ps[:],
                        axis=mybir.AxisListType.X,
                    )
                    slot[b] += 1
                else:
                    if b in halves:
                        # complete the pair tile
                        pair = halves.pop(b)
                        nc.scalar.activation(
                            pair[:, 1024:2048], ps[:], AF.Copy, scale=1.0
                        )
                        if b not in accs:
                            accs[b] = pair
                        else:
                            nc.vector.tensor_tensor(
                                accs[b][:], accs[b][:], pair[:], op=ALU.max
                            )
                    elif n_staged % 2 == 1 and b not in odd1 and b in accs:
                        # odd leftover: single [128,1024] staged piece
                        st1 = s1pool.tile([128, PIECE], f16, tag="st1")
                        nc.scalar.activation(st1[:], ps[:], AF.Copy, scale=1.0)
                        odd1[b] = st1
                    else:
                        tile_src = apool if b not in accs else stpool
                        pair = tile_src.tile([128, 2048], f16, tag="pair",
                                             name=f"pair{b}_{p}")
                        nc.scalar.activation(
                            pair[:, 0:1024], ps[:], AF.Copy, scale=1.0
                        )
                        halves[b] = pair
                if p == NPIECE - 1:
                    acc = accs.pop(b)
                    # fold halves, then leftover, then one [128,1024] reduce
                    nc.vector.tensor_tensor(
                        acc[:, 0:1024], acc[:, 0:1024], acc[:, 1024:2048],
                        op=ALU.max,
                    )
                    if b in odd1:
                        nc.vector.tensor_tensor(
                            acc[:, 0:1024], acc[:, 0:1024], odd1.pop(b)[:],
                            op=ALU.max,
                        )
                    nc.vector.reduce_max(
                        mstage[:, b * SLOTS + slot[b]:b * SLOTS + slot[b] + 1],
                        acc[:, 0:1024],
                        axis=mybir.AxisListType.X,
                    )
                    slot[b] += 1
                if (b, p) == (0, 8):
                    # positives: elementwise r_i * r_{i+B} (cols 0..2047 vs
                    # 8192..10239 of the rotated layout)
                    nc.vector.tensor_tensor(posP[:], R[0][:], R[4][:], op=ALU.mult)

            # ---- positives: partition-sum via ones-column matmul, then exp
            for half in range(2):
                ps_pos = psumpool.tile([128, PIECE], f32, tag="ps")
                for t in range(2):
                    off = half * 1024 + t * 512
                    nc.tensor.matmul(
                        ps_pos[:, t * 512:(t + 1) * 512],
                        wcol[:],
                        posP[:, off:off + 512],
                        start=True,
                        stop=True,
                    )
                pxp = cpool.tile([1, PIECE], f32, tag=f"posexp{half}",
                                 name=f"posexp{half}")
                nc.scalar.activation(
                    pxp[:], ps_pos[0:1, :], AF.Exp, scale=1.0 / TEMP,
                    accum_out=outstage[0:1, 1 + half:2 + half],
                )

            # ---- finalize: block maxes -> lse = exp(max/T) ----
            bmax = cpool.tile([128, BLKS], f32, tag="bmax")
            nc.vector.reduce_max(
                bmax[:],
                mstage[:].rearrange("p (b s) -> p b s", s=SLOTS),
                axis=mybir.AxisListType.X,
            )
            lscr = cpool.tile([128, BLKS], f32, tag="lscr")
            nc.scalar.activation(
                lscr[:], bmax[:], AF.Exp, scale=1.0 / TEMP,
                accum_out=outstage[:, 0:1],
            )
            nc.sync.dma_start(
                out_d.rearrange("(p o) -> p o", o=4),
                outstage[:],
            )

    nc.compile()
    return nc


def make_in_maps(z_i: np.ndarray, z_j: np.ndarray):
    Z = np.concatenate([np.asarray(z_i), np.asarray(z_j)], axis=0).astype(np.float32)
    nrm = np.linalg.norm(Z, axis=1, keepdims=True)
    R = (Z / np.maximum(nrm, 1e-12)).astype(np.float32)
    RT = np.ascontiguousarray(R.T).astype(np.float16)  # [128, 16384]
    eyew = np.eye(128, dtype=np.float16)
    m4 = np.zeros((128, 2048), dtype=np.float16)
    for k in range(4):
        for p in range(128):
            m4[p, 512 * k + 128 * k + p] = -30000.0
    wcol = np.zeros((128, 128), dtype=np.float16)
    wcol[:, 0] = 1.0
    in_maps = []
    for c in range(NCORES):
        zt = np.ascontiguousarray(np.roll(RT, -c * ROWS_PER_CORE, axis=1))
        in_maps.append({"zt": zt, "eyew": eyew, "m4": m4, "wcol": wcol})
    return in_maps


def kernel(z_i: np.ndarray, z_j: np.ndarray) -> np.ndarray:
    from concourse.bass_utils import run_bass_kernel_spmd

    if "nc" not in _cache:
        _cache["nc"] = build_nc()
    nc = _cache["nc"]

    in_maps = make_in_maps(z_i, z_j)
    res = run_bass_kernel_spmd(
        nc,
        in_maps,
        core_ids=list(range(NCORES)),
        trace=bool(int(os.environ.get("NTX_TRACE", "0"))),
    )
    _cache["last_result"] = res

    lse_sum = 0.0
    pos_sum = 0.0
    for c in range(NCORES):
        out = res.results[c]["out"].astype(np.float64).reshape(128, 4)
        lse_sum += out[:, 0].sum()
        pos_sum += out[:, 1].sum() + out[:, 2].sum()
    loss = (lse_sum - pos_sum) / float(N)
    return np.float32(loss)
